# revision 27
# baseline (speedup 1.0000x reference)
"""LlamaAttention (B=1, S=2048, H=4096, 32 q-heads / 8 kv-heads, hd=128) on 8 trn2 cores.

Sharding: tensor-parallel over heads. Core c owns q-heads 4c..4c+3 and kv-head c
(GQA group == 4 aligns exactly). Pipeline v2:
  - hidden is pre-transposed + bf16-cast on host (hT chunk/partition-major), so the
    kernel does no PE transposes for the QKV input and input DMA traffic is halved.
  - Per s-chunk qc (512 q): QKV GEMM (m order K,Q0..Q3,V) -> RoPE -> attention(qc)
    -> chunked AllGather of O^T -> o_proj(qc) interleaved into the PE stream as
    Q0 A0 Q1 A1 O0 Q2 A2 O1 Q3 A3 O2 O3 so collectives/DMA hide under compute.
  - Attention in S^T layout: S^T[k,q] = K'^T Q', exp (no max-sub; scores bounded),
    multiplicative triangular mask, then PV is flipped: O^T[d,q] = V[k,d]^T P^T[k,q]
    with N=512 streams (no short-N matmuls, no output transposes). The softmax
    denominator comes from a DVE reduce over k-tiles + a ones-matmul partition
    broadcast; normalization is fused into the PSUM evict.
  - Causal rectangle restriction: diagonal k-tiles only compute/exp/accumulate
    columns >= 128*v (es slab pre-zeroed once).
  - o_proj k-major with 4 concurrent PSUM chains; og streamed per k-tile from the
    AllGather result (8-tile rotating pool).
Host side does sharding/unsharding, dtype casts, layout transforms and structural
constants (identity, R, ones, triangular masks, cos/sin from positions).
"""

import numpy as np
import ml_dtypes
from contextlib import ExitStack

import concourse.bass as bass
import concourse.tile as tile
from concourse import bacc, mybir
from concourse.bass_utils import run_bass_kernel_spmd

BF16 = mybir.dt.bfloat16
F32 = mybir.dt.float32
NPBF16 = ml_dtypes.bfloat16

S = 2048
H = 4096
NH, NKV, HD = 32, 8, 128
NCORES = 8
QH = NH // NCORES            # 4 q heads per core
FC = (QH + 2) * HD           # 768 qkv columns per core
WON = H // NCORES            # 512 o_proj output columns per core
P = 128
CH = 512                     # s-chunk
NCH = S // CH                # 4 chunks
KT = S // P                  # 16 k tiles
HT = H // P                  # 32 h tiles
HG = 8                       # hT DMA groups per chunk (4 ktiles each)
SCALE = 1.0 / float(np.sqrt(HD))

_CACHE: dict = {}


def _emit(nc: bass.Bass, tc, aps):
    ctx = ExitStack()
    hTd, wqd, wod, cosT, sinT, rT, ident, ones, tri, out = (
        aps["hT"], aps["wq"], aps["wo"], aps["cosT"], aps["sinT"],
        aps["rT"], aps["ident"], aps["ones"], aps["tri"], aps["out"],
    )

    # ---------------- pools ----------------
    const = ctx.enter_context(tc.tile_pool(name="const", bufs=1))
    cos_sb = const.tile([P, S], BF16)
    sin_sb = const.tile([P, S], BF16)
    rT_sb = const.tile([P, P], BF16)
    id_sb = const.tile([P, P], BF16)
    ones_sb = const.tile([P, P], BF16)
    tri_sb = const.tile([P, 4, CH], BF16)

    persist = ctx.enter_context(tc.tile_pool(name="persist", bufs=1))
    kp = persist.tile([P, S], BF16)          # K'^T (rope'd), [d, s]
    vt = persist.tile([P, KT, P], BF16)      # V [s-part, ktile, d]
    es = persist.tile([P, KT, CH], BF16)     # exp(scores^T) for current head
    wq_sb = persist.tile([P, HG, 6, 4, P], BF16)  # qkv weights, k-group-major
    wo_sb = persist.tile([P, HT, WON], BF16)

    qp_pool = ctx.enter_context(tc.tile_pool(name="qp", bufs=1))
    hT_pool = ctx.enter_context(tc.tile_pool(name="hT", bufs=HG + 4))
    og_pool = ctx.enter_context(tc.tile_pool(name="og", bufs=4))
    ot_pool = ctx.enter_context(tc.tile_pool(name="ot", bufs=2))
    tmp_pool = ctx.enter_context(tc.tile_pool(name="tmp", bufs=2))
    den_pool = ctx.enter_context(tc.tile_pool(name="den", bufs=2))
    oev_pool = ctx.enter_context(tc.tile_pool(name="oev", bufs=2))

    chain_pool = ctx.enter_context(tc.tile_pool(name="chain", bufs=2, space="PSUM"))
    pss_pool = ctx.enter_context(tc.tile_pool(name="pss", bufs=2, space="PSUM"))
    acc_pool = ctx.enter_context(tc.tile_pool(name="acc", bufs=4, space="PSUM"))

    dram = ctx.enter_context(tc.tile_pool(name="dram", bufs=1, space="DRAM"))
    ag_ins = [
        (dram.tile([2 * P, CH], BF16, name=f"agia{qc}"),
         dram.tile([2 * P, CH], BF16, name=f"agib{qc}"))
        for qc in range(NCH)
    ]
    ag_outs = [
        (dram.tile([H // 2, CH], BF16, addr_space="Shared", name=f"agoa{qc}"),
         dram.tile([H // 2, CH], BF16, addr_space="Shared", name=f"agob{qc}"))
        for qc in range(NCH)
    ]

    # ---------------- startup DMA ----------------
    # gpsimd(SWDGE): weights / ag_in / out / collectives.
    # sync(HWDGE):   hT group stream.
    # scalar(HWDGE): constants early, og ktile stream later.
    # wq per k-group, alternating rings, in chunk-0 consumption order.
    # The first groups are split by m-block (K's m=4 lands first).
    for g in range(HG):
        eng = nc.gpsimd if g % 2 == 0 else nc.scalar
        if g < 2:
            eng.dma_start(wq_sb[:, g, 4:6, :, :], wqd[:, g, 4:6, :, :])
            eng.dma_start(wq_sb[:, g, 0:4, :, :], wqd[:, g, 0:4, :, :])
        else:
            eng.dma_start(wq_sb[:, g, :, :, :], wqd[:, g, :, :, :])
    nc.scalar.dma_start(cos_sb[:], cosT[:])
    nc.scalar.dma_start(sin_sb[:], sinT[:])
    nc.scalar.dma_start(rT_sb[:], rT[:])
    nc.scalar.dma_start(id_sb[:], ident[:])
    nc.scalar.dma_start(ones_sb[:], ones[:])
    nc.scalar.dma_start(tri_sb[:], tri[:])
    nc.scalar.dma_start(wo_sb[:], wod[:])
    # collective warm-up so the first real AllGather doesn't pay ncfw cold start;
    # placed after the startup-critical weight DMAs (the trigger holds the ring).
    warm_dram = ctx.enter_context(tc.tile_pool(name="warm_dram", bufs=1, space="DRAM"))
    warm_in = warm_dram.tile([P, 4], BF16)
    warm_out = warm_dram.tile([NCORES * P, 4], BF16, addr_space="Shared")
    warm_sb = const.tile([P, 4], BF16)
    nc.vector.memset(warm_sb[:], 0.0)
    nc.gpsimd.dma_start(warm_in[:], warm_sb[:])
    nc.gpsimd.collective_compute(
        "AllGather",
        mybir.AluOpType.bypass,
        ins=[warm_in.opt()],
        outs=[warm_out.opt()],
        replica_groups=[list(range(NCORES))],
    )

    # es must start zeroed so diagonal-tile rectangle writes leave masked cols 0
    nc.vector.memset(es[:], 0.0)

    MORDER = (4, 0, 1, 2, 3, 5)  # K first (attention needs it), V last

    def qkv_chunk(qc):
        sl = slice(qc * CH, (qc + 1) * CH)
        hts = []
        for g in range(HG):
            ht = hT_pool.tile([P, 4, CH], BF16, name="ht")
            if qc == 0 and g < 2:
                nc.sync.dma_start(ht[:, 0:2, :], hTd[:, qc, g * 4:g * 4 + 2, :])
                nc.sync.dma_start(ht[:, 2:4, :], hTd[:, qc, g * 4 + 2:g * 4 + 4, :])
            else:
                nc.sync.dma_start(ht[:], hTd[:, qc, g * 4:(g + 1) * 4, :])
            hts.append(ht)
        qp = qp_pool.tile([P, QH, CH], BF16, name="qp")

        # RoPE stage 2+: emitted one chain late so the PE never waits on the
        # ACT psum evict of the chain it just finished.
        def rope_finish(m, raw):
            psr = pss_pool.tile([P, CH], F32, name="psr", tag="s")
            nc.tensor.matmul(psr[:], rT_sb[:], raw[:], start=True, stop=True)
            rot = tmp_pool.tile([P, CH], BF16, name="rot")
            nc.vector.tensor_copy(rot[:], psr[:])
            t1 = tmp_pool.tile([P, CH], BF16, name="t1")
            nc.vector.tensor_mul(t1[:], raw[:], cos_sb[:, sl])
            nc.vector.tensor_mul(rot[:], rot[:], sin_sb[:, sl])
            dst = kp[:, sl] if m == 4 else qp[:, m, :]
            nc.vector.tensor_add(dst, t1[:], rot[:])

        if qc == 0:
            # chunk 0 is DMA-bound: interleave all 6 chains k-group-major over
            # 6 PSUM tiles so the PE consumes each arriving hT/wq group 6x
            # slower and never outruns the cold DMA stream.
            ps6 = {}
            for m in range(4):
                ps6[m] = acc_pool.tile([P, CH], F32, name="psv", tag="a")
            ps6[4] = chain_pool.tile([P, CH], F32, name="psq", tag="c")
            ps6[5] = chain_pool.tile([P, CH], F32, name="psq", tag="c")
            for g in range(HG):
                for m in MORDER:
                    for kj in range(4):
                        nc.tensor.matmul(
                            ps6[m][:],
                            wq_sb[:, g, m, kj, :],
                            hts[g][:, kj, :],
                            start=(g == 0 and kj == 0),
                            stop=(g == HG - 1 and kj == 3),
                        )
            for m in MORDER:
                ps = ps6[m]
                if m == 5:
                    vraw = tmp_pool.tile([P, CH], BF16, name="vraw")
                    nc.scalar.copy(vraw[:], ps[:])
                    psv = pss_pool.tile([P, CH], F32, name="psr", tag="s")
                    psvb = psv.bitcast(BF16)
                    for t in range(4):
                        nc.tensor.transpose(
                            psvb[:, t * P:(t + 1) * P], vraw[:, t * P:(t + 1) * P],
                            id_sb[:],
                        )
                        nc.vector.tensor_copy(
                            vt[:, t, :], psvb[:, t * P:(t + 1) * P]
                        )
                else:
                    raw = tmp_pool.tile([P, CH], BF16, name="raw")
                    nc.scalar.copy(raw[:], ps[:])
                    rope_finish(m, raw)
            qkv_chunk.qp = qp
            return

        pending = None
        for m in MORDER:
            ps = chain_pool.tile([P, CH], F32, name="psq", tag="c")
            for k in range(HT):
                nc.tensor.matmul(
                    ps[:],
                    wq_sb[:, k // 4, m, k % 4, :],
                    hts[k // 4][:, k % 4, :],
                    start=(k == 0),
                    stop=(k == HT - 1),
                )
            if pending is not None:
                rope_finish(*pending)
                pending = None
            if m == 5:
                # V^T chunk -> V tiles [s, d] via PE transpose
                vraw = tmp_pool.tile([P, CH], BF16, name="vraw")
                nc.scalar.copy(vraw[:], ps[:])
                psv = acc_pool.tile([P, CH], F32, name="psv", tag="a")
                psvb = psv.bitcast(BF16)
                for t in range(4):
                    nc.tensor.transpose(
                        psvb[:, t * P:(t + 1) * P], vraw[:, t * P:(t + 1) * P], id_sb[:]
                    )
                    nc.vector.tensor_copy(vt[:, 4 * qc + t, :], psvb[:, t * P:(t + 1) * P])
            else:
                raw = tmp_pool.tile([P, CH], BF16, name="raw")
                nc.scalar.copy(raw[:], ps[:])
                pending = (m, raw)
        qkv_chunk.qp = qp

    ot_cur = [None]

    def attn_chunk(qc, heads=range(QH)):
        nk = 4 * qc + 4
        qp = qkv_chunk.qp
        if heads[0] == 0:
            ot_cur[0] = ot_pool.tile([P, QH, CH], BF16, name="ot")
        ot = ot_cur[0]
        pvs, denbs = [], []
        for h in heads:
            # scores + exp (+ causal mask on diagonal tiles)
            for kj in range(nk):
                v = kj - 4 * qc
                c0 = 128 * v if v >= 0 else 0
                pss = pss_pool.tile([P, CH], F32, name="pss", tag="s")
                nc.tensor.matmul(
                    pss[:, c0:CH],
                    kp[:, kj * P:(kj + 1) * P],
                    qp[:, h, c0:CH],
                    start=True,
                    stop=True,
                )
                nc.scalar.activation(
                    es[:, kj, c0:CH], pss[:, c0:CH],
                    mybir.ActivationFunctionType.Exp,
                    scale=SCALE,
                )
                if v >= 0:
                    nc.vector.tensor_mul(
                        es[:, kj, c0:CH], es[:, kj, c0:CH], tri_sb[:, v, c0:CH]
                    )
            # denominator: pairwise bf16 tree over full es tiles (masked regions
            # are exactly zero, so full-tile sums are correct). bf16 runs 2x the
            # DVE rate of f32; the ~4-level rounding noise (~0.5%) fits budget.
            denb = tmp_pool.tile([P, CH], BF16, name="denb", bufs=QH)

            def tt_add(a, b, final=False):
                if final:
                    nc.vector.tensor_add(denb[:], a, b)
                    return denb[:]
                t = den_pool.tile([P, CH], BF16, name="tt", bufs=6)
                nc.vector.tensor_add(t[:], a, b)
                return t[:]

            # quad-group running ladder: every tt is consumed within <=4 DVE
            # ops of its write (in-order queue safe), ~3 rounding levels
            ngr = nk // 4
            acc = None
            for g in range(ngr):
                l = [es[:, 4 * g + j, :] for j in range(4)]
                p1 = tt_add(l[0], l[1])
                p2 = tt_add(l[2], l[3])
                gsum = tt_add(p1, p2, final=(ngr == 1))
                if g == 0:
                    acc = gsum
                else:
                    acc = tt_add(acc, gsum, final=(g == ngr - 1))
            denbs.append((h, denb))
            # flipped PV: O^T[d, q] accumulated over k tiles (masked es)
            pv = acc_pool.tile([P, CH], F32, name="pv", tag="a")
            for kj in range(nk):
                v = kj - 4 * qc
                c0 = 128 * v if v > 0 else 0
                nc.tensor.matmul(
                    pv[:, c0:CH],
                    vt[:, kj, :],
                    es[:, kj, c0:CH],
                    start=(kj == 0),
                    stop=(kj == nk - 1),
                )
            pvs.append((h, pv))
            if h % 2 == 1:
                # normalize + ship this head pair now (deferred past the pair's
                # PV so the PE never waits the DVE denominator chain), keeping
                # the pair's AllGather as early as possible for o_proj
                pair = h // 2
                dd = dict(denbs)
                pp = dict(pvs)
                for hh in (h - 1, h):
                    dps = chain_pool.tile([P, CH], F32, name="dps", tag="c")
                    nc.tensor.matmul(
                        dps[:], ones_sb[:], dd[hh][:], start=True, stop=True
                    )
                    rec = den_pool.tile([P, CH], F32, name="rec", bufs=1)
                    nc.vector.reciprocal_approx_fast(rec[:], dps[:])
                    nc.vector.tensor_mul(ot[:, hh, :], pp[hh][:], rec[:])
                    nc.gpsimd.dma_start(
                        ag_ins[qc][pair][(hh % 2) * P:(hh % 2 + 1) * P, :],
                        ot[:, hh, :],
                    )
                nc.gpsimd.collective_compute(
                    "AllGather",
                    mybir.AluOpType.bypass,
                    ins=[ag_ins[qc][pair].opt()],
                    outs=[ag_outs[qc][pair].opt()],
                    replica_groups=[list(range(NCORES))],
                )

    ogs_fetched = {}

    def og_fetch(qc):
        # og k-pair stream on the gpsimd(SWDGE) ring. Emitted during the NEXT
        # qkv chunk (ring position after that chunk's AllGather triggers), so
        # all pairs land before oproj(qc) starts and nothing inverts in-order.
        agr_a = ag_outs[qc][0].rearrange("(k p) q -> p k q", p=P)
        agr_b = ag_outs[qc][1].rearrange("(k p) q -> p k q", p=P)
        ogs = []
        for kp2 in range(HT // 2):
            og = og_pool.tile([P, 2, CH], BF16, name="og")
            k2 = 2 * kp2
            src_ap = (agr_a[:, k2:k2 + 2, :] if k2 < KT
                      else agr_b[:, k2 - KT:k2 - KT + 2, :])
            nc.gpsimd.dma_start(og[:], src_ap)
            ogs.append(og[:, 0, :])
            ogs.append(og[:, 1, :])
        ogs_fetched[qc] = ogs

    def oproj_chunk(qc):
        ogs = ogs_fetched[qc]
        psos = [acc_pool.tile([P, WON], F32, name="pso", tag="a") for _ in range(4)]
        for k in range(HT):
            for mi in range(4):
                nc.tensor.matmul(
                    psos[mi][:],
                    ogs[k][:, mi * P:(mi + 1) * P],
                    wo_sb[:, k, :],
                    start=(k == 0),
                    stop=(k == HT - 1),
                )
        for mi in range(4):
            m = qc * 4 + mi
            oev = oev_pool.tile([P, WON], BF16, name="oev")
            nc.vector.tensor_copy(oev[:], psos[mi][:])
            # scalar ring: idle while o_proj runs, so the final chunk's output
            # isn't stuck behind the og trigger burst on the gpsimd ring
            nc.scalar.dma_start(out[m * P:(m + 1) * P, :], oev[:])

    # ---------------- pipelined schedule ----------------
    qkv_chunk(0)
    attn_chunk(0)
    qkv_chunk(1)
    og_fetch(0)
    attn_chunk(1)
    oproj_chunk(0)
    qkv_chunk(2)
    og_fetch(1)
    attn_chunk(2)
    oproj_chunk(1)
    qkv_chunk(3)
    og_fetch(2)
    attn_chunk(3, heads=(0, 1))
    oproj_chunk(2)
    attn_chunk(3, heads=(2, 3))
    og_fetch(3)
    oproj_chunk(3)

    ctx.close()


def _build():
    if "nc" in _CACHE:
        return _CACHE["nc"]
    nc = bacc.Bacc("TRN2", debug=False, num_devices=NCORES, target_bir_lowering=False)
    aps = {}
    aps["hT"] = nc.dram_tensor("hT", [P, NCH, HT, CH], BF16, kind="ExternalInput").ap()
    aps["wq"] = nc.dram_tensor("wq", [P, HG, 6, 4, P], BF16, kind="ExternalInput").ap()
    aps["wo"] = nc.dram_tensor("wo", [P, HT, WON], BF16, kind="ExternalInput").ap()
    aps["cosT"] = nc.dram_tensor("cosT", [HD, S], BF16, kind="ExternalInput").ap()
    aps["sinT"] = nc.dram_tensor("sinT", [HD, S], BF16, kind="ExternalInput").ap()
    aps["rT"] = nc.dram_tensor("rT", [P, P], BF16, kind="ExternalInput").ap()
    aps["ident"] = nc.dram_tensor("ident", [P, P], BF16, kind="ExternalInput").ap()
    aps["ones"] = nc.dram_tensor("ones", [P, P], BF16, kind="ExternalInput").ap()
    aps["tri"] = nc.dram_tensor("tri", [P, 4, CH], BF16, kind="ExternalInput").ap()
    aps["out"] = nc.dram_tensor("out", [S, WON], BF16, kind="ExternalOutput").ap()
    with tile.TileContext(nc) as tc:
        _emit(nc, tc, aps)
    nc.compile()
    _CACHE["nc"] = nc
    return nc


def _host_tables(positions: np.ndarray):
    pos = np.asarray(positions).reshape(-1).astype(np.float64)
    assert pos.shape[0] == S
    inv = 1.0 / (10000.0 ** (np.arange(0, HD, 2, dtype=np.float64) / HD))  # [64]
    invf = np.concatenate([inv, inv])  # [128], row d uses inv[d % 64]
    th = invf[:, None] * pos[None, :]  # [128, 2048]
    cosT = np.cos(th).astype(NPBF16)
    sinT = np.sin(th).astype(NPBF16)
    R = np.zeros((P, P), np.float32)
    idx = np.arange(64)
    R[idx, idx + 64] = -1.0
    R[idx + 64, idx] = 1.0
    rT = R.T.astype(NPBF16).copy()
    ident = np.eye(P, dtype=NPBF16)
    ones = np.ones((P, P), dtype=NPBF16)
    k_loc = np.arange(P)[:, None]
    q_loc = np.arange(CH)[None, :]
    tri = np.stack(
        [(q_loc >= k_loc + 128 * v) for v in range(4)], axis=1
    ).astype(NPBF16)  # [128, 4, 512]
    return cosT, sinT, rT, ident, ones, tri


def _make_in_maps(inputs: dict):
    hidden = np.asarray(inputs["hidden_states"], np.float32).reshape(S, H)
    positions = np.asarray(inputs["positions"])
    w_qkv = np.asarray(inputs["w_qkv"], np.float32)
    w_o = np.asarray(inputs["w_o"], np.float32)
    cosT, sinT, rT, ident, ones, tri = _host_tables(positions)
    # hT: [4096, 2048] -> [p=128, chunk=4, ktile=32, s=512] (contiguous per partition)
    hT = np.ascontiguousarray(
        hidden.T.astype(NPBF16)
        .reshape(HT, P, NCH, CH)
        .transpose(1, 2, 0, 3)
    )
    in_maps = []
    for c in range(NCORES):
        wqc = np.concatenate([
            w_qkv[:, c * QH * HD:(c + 1) * QH * HD],
            w_qkv[:, NH * HD + c * HD: NH * HD + (c + 1) * HD],
            w_qkv[:, (NH + NKV) * HD + c * HD: (NH + NKV) * HD + (c + 1) * HD],
        ], axis=1)  # [4096, 768] cols: Q0..Q3, K, V
        # -> [p=128, kgroup=8, m=6, kj=4, 128] (k-group-major so chunk-0's
        # group-interleaved chains consume weights in DMA arrival order)
        wq = np.ascontiguousarray(
            wqc.astype(NPBF16).reshape(8, 4, P, 6, P).transpose(2, 0, 3, 1, 4)
        )
        # feature order after the two head-pair AllGathers: ktile k<16 holds
        # (core k//2, head k%2); k>=16 holds (core (k-16)//2, head 2+(k-16)%2)
        heads = [4 * (k // 2) + (k % 2) for k in range(KT)] + \
                [4 * (k // 2) + 2 + (k % 2) for k in range(KT)]
        perm = np.concatenate([h * HD + np.arange(HD) for h in heads])
        wo = np.ascontiguousarray(
            w_o[perm, c * WON:(c + 1) * WON].astype(NPBF16)
            .reshape(HT, P, WON).transpose(1, 0, 2)
        )
        in_maps.append({
            "hT": hT,
            "wq": wq,
            "wo": wo,
            "cosT": cosT,
            "sinT": sinT,
            "rT": rT,
            "ident": ident,
            "ones": ones,
            "tri": tri,
        })
    return in_maps


def _run(inputs: dict, trace: bool = False):
    nc = _build()
    in_maps = _make_in_maps(inputs)
    res = run_bass_kernel_spmd(nc, in_maps, core_ids=list(range(NCORES)), trace=trace)
    full = np.concatenate(
        [np.asarray(res.results[c]["out"]) for c in range(NCORES)], axis=1
    )
    return full.reshape(1, S, H).astype(np.float32), res


def kernel(**inputs) -> np.ndarray:
    out, _ = _run(inputs, trace=False)
    return out


if __name__ == "__main__":
    import sys
    if "--build-only" in sys.argv:
        nc = _build()
        print("build ok; instructions:",
              sum(len(bb.instructions) for bb in nc.main_func.blocks))


# revision 28
# speedup vs baseline: 1.0331x; 1.0331x over previous
"""LlamaAttention (B=1, S=2048, H=4096, 32 q-heads / 8 kv-heads, hd=128) on 8 trn2 cores.

Sharding: tensor-parallel over heads. Core c owns q-heads 4c..4c+3 and kv-head c
(GQA group == 4 aligns exactly). Pipeline v2:
  - hidden is pre-transposed + bf16-cast on host (hT chunk/partition-major), so the
    kernel does no PE transposes for the QKV input and input DMA traffic is halved.
  - Per s-chunk qc (512 q): QKV GEMM (m order K,Q0..Q3,V) -> RoPE -> attention(qc)
    -> chunked AllGather of O^T -> o_proj(qc) interleaved into the PE stream as
    Q0 A0 Q1 A1 O0 Q2 A2 O1 Q3 A3 O2 O3 so collectives/DMA hide under compute.
  - Attention in S^T layout: S^T[k,q] = K'^T Q', exp (no max-sub; scores bounded),
    multiplicative triangular mask, then PV is flipped: O^T[d,q] = V[k,d]^T P^T[k,q]
    with N=512 streams (no short-N matmuls, no output transposes). The softmax
    denominator comes from a DVE reduce over k-tiles + a ones-matmul partition
    broadcast; normalization is fused into the PSUM evict.
  - Causal rectangle restriction: diagonal k-tiles only compute/exp/accumulate
    columns >= 128*v (es slab pre-zeroed once).
  - o_proj k-major with 4 concurrent PSUM chains; og streamed per k-tile from the
    AllGather result (8-tile rotating pool).
Host side does sharding/unsharding, dtype casts, layout transforms and structural
constants (identity, R, ones, triangular masks, cos/sin from positions).
"""

import numpy as np
import ml_dtypes
from contextlib import ExitStack

import concourse.bass as bass
import concourse.tile as tile
from concourse import bacc, mybir
from concourse.bass_utils import run_bass_kernel_spmd

BF16 = mybir.dt.bfloat16
F32 = mybir.dt.float32
NPBF16 = ml_dtypes.bfloat16

S = 2048
H = 4096
NH, NKV, HD = 32, 8, 128
NCORES = 8
QH = NH // NCORES            # 4 q heads per core
FC = (QH + 2) * HD           # 768 qkv columns per core
WON = H // NCORES            # 512 o_proj output columns per core
P = 128
CH = 512                     # s-chunk
NCH = S // CH                # 4 chunks
KT = S // P                  # 16 k tiles
HT = H // P                  # 32 h tiles
HG = 8                       # hT DMA groups per chunk (4 ktiles each)
SCALE = 1.0 / float(np.sqrt(HD))

_CACHE: dict = {}


def _emit(nc: bass.Bass, tc, aps):
    ctx = ExitStack()
    hTd, wqd, wod, cosT, sinT, rT, ident, ones, tri, out = (
        aps["hT"], aps["wq"], aps["wo"], aps["cosT"], aps["sinT"],
        aps["rT"], aps["ident"], aps["ones"], aps["tri"], aps["out"],
    )

    # ---------------- pools ----------------
    const = ctx.enter_context(tc.tile_pool(name="const", bufs=1))
    cos_sb = const.tile([P, S], BF16)
    sin_sb = const.tile([P, S], BF16)
    rT_sb = const.tile([P, P], BF16)
    id_sb = const.tile([P, P], BF16)
    ones_sb = const.tile([P, P], BF16)
    tri_sb = const.tile([P, 4, CH], BF16)

    persist = ctx.enter_context(tc.tile_pool(name="persist", bufs=1))
    kp = persist.tile([P, S], BF16)          # K'^T (rope'd), [d, s]
    vt = persist.tile([P, KT, P], BF16)      # V [s-part, ktile, d]
    es = persist.tile([P, KT, CH], BF16)     # exp(scores^T) for current head
    wq_sb = persist.tile([P, HG, 6, 4, P], BF16)  # qkv weights, k-group-major
    wo_sb = persist.tile([P, HT, WON], BF16)

    qp_pool = ctx.enter_context(tc.tile_pool(name="qp", bufs=1))
    hT_pool = ctx.enter_context(tc.tile_pool(name="hT", bufs=HG + 4))
    og_pool = ctx.enter_context(tc.tile_pool(name="og", bufs=4))
    ot_pool = ctx.enter_context(tc.tile_pool(name="ot", bufs=2))
    tmp_pool = ctx.enter_context(tc.tile_pool(name="tmp", bufs=2))
    den_pool = ctx.enter_context(tc.tile_pool(name="den", bufs=2))
    oev_pool = ctx.enter_context(tc.tile_pool(name="oev", bufs=2))

    chain_pool = ctx.enter_context(tc.tile_pool(name="chain", bufs=2, space="PSUM"))
    pss_pool = ctx.enter_context(tc.tile_pool(name="pss", bufs=2, space="PSUM"))
    acc_pool = ctx.enter_context(tc.tile_pool(name="acc", bufs=4, space="PSUM"))

    dram = ctx.enter_context(tc.tile_pool(name="dram", bufs=1, space="DRAM"))
    ag_ins = [
        (dram.tile([2 * P, CH], BF16, name=f"agia{qc}"),
         dram.tile([2 * P, CH], BF16, name=f"agib{qc}"))
        for qc in range(NCH)
    ]
    ag_outs = [
        (dram.tile([H // 2, CH], BF16, addr_space="Shared", name=f"agoa{qc}"),
         dram.tile([H // 2, CH], BF16, addr_space="Shared", name=f"agob{qc}"))
        for qc in range(NCH)
    ]

    # ---------------- startup DMA ----------------
    # gpsimd(SWDGE): weights / ag_in / out / collectives.
    # sync(HWDGE):   hT group stream.
    # scalar(HWDGE): constants early, og ktile stream later.
    # wq per k-group, alternating rings, in chunk-0 consumption order.
    # The first groups are split by m-block (K's m=4 lands first).
    for g in range(HG):
        eng = nc.gpsimd if g % 2 == 0 else nc.scalar
        if g < 2:
            eng.dma_start(wq_sb[:, g, 4:6, :, :], wqd[:, g, 4:6, :, :])
            eng.dma_start(wq_sb[:, g, 0:4, :, :], wqd[:, g, 0:4, :, :])
        else:
            eng.dma_start(wq_sb[:, g, :, :, :], wqd[:, g, :, :, :])
    nc.scalar.dma_start(cos_sb[:], cosT[:])
    nc.scalar.dma_start(sin_sb[:], sinT[:])
    nc.scalar.dma_start(rT_sb[:], rT[:])
    nc.scalar.dma_start(id_sb[:], ident[:])
    nc.scalar.dma_start(ones_sb[:], ones[:])
    nc.scalar.dma_start(tri_sb[:], tri[:])
    nc.scalar.dma_start(wo_sb[:], wod[:])
    # collective warm-up so the first real AllGather doesn't pay ncfw cold start;
    # placed after the startup-critical weight DMAs (the trigger holds the ring).
    warm_dram = ctx.enter_context(tc.tile_pool(name="warm_dram", bufs=1, space="DRAM"))
    warm_in = warm_dram.tile([P, 4], BF16)
    warm_out = warm_dram.tile([NCORES * P, 4], BF16, addr_space="Shared")
    warm_sb = const.tile([P, 4], BF16)
    nc.vector.memset(warm_sb[:], 0.0)
    nc.gpsimd.dma_start(warm_in[:], warm_sb[:])
    nc.gpsimd.collective_compute(
        "AllGather",
        mybir.AluOpType.bypass,
        ins=[warm_in.opt()],
        outs=[warm_out.opt()],
        replica_groups=[list(range(NCORES))],
    )

    # es must start zeroed so diagonal-tile rectangle writes leave masked cols 0
    nc.vector.memset(es[:], 0.0)

    MORDER = (4, 0, 1, 2, 3, 5)  # K first (attention needs it), V last

    def qkv_chunk(qc):
        sl = slice(qc * CH, (qc + 1) * CH)
        hts = []
        for g in range(HG):
            ht = hT_pool.tile([P, 4, CH], BF16, name="ht")
            if qc == 0 and g < 2:
                nc.sync.dma_start(ht[:, 0:2, :], hTd[:, qc, g * 4:g * 4 + 2, :])
                nc.sync.dma_start(ht[:, 2:4, :], hTd[:, qc, g * 4 + 2:g * 4 + 4, :])
            else:
                nc.sync.dma_start(ht[:], hTd[:, qc, g * 4:(g + 1) * 4, :])
            hts.append(ht)
        qp = qp_pool.tile([P, QH, CH], BF16, name="qp")

        # RoPE stage 2+: emitted one chain late so the PE never waits on the
        # ACT psum evict of the chain it just finished.
        def rope_finish(m, raw):
            psr = pss_pool.tile([P, CH], F32, name="psr", tag="s")
            nc.tensor.matmul(psr[:], rT_sb[:], raw[:], start=True, stop=True)
            rot = tmp_pool.tile([P, CH], BF16, name="rot")
            nc.vector.tensor_copy(rot[:], psr[:])
            t1 = tmp_pool.tile([P, CH], BF16, name="t1")
            nc.vector.tensor_mul(t1[:], raw[:], cos_sb[:, sl])
            nc.vector.tensor_mul(rot[:], rot[:], sin_sb[:, sl])
            dst = kp[:, sl] if m == 4 else qp[:, m, :]
            nc.vector.tensor_add(dst, t1[:], rot[:])

        if qc == 0:
            # chunk 0 is DMA-bound: interleave all 6 chains k-group-major over
            # 6 PSUM tiles so the PE consumes each arriving hT/wq group 6x
            # slower and never outruns the cold DMA stream.
            ps6 = {}
            for m in range(4):
                ps6[m] = acc_pool.tile([P, CH], F32, name="psv", tag="a")
            ps6[4] = chain_pool.tile([P, CH], F32, name="psq", tag="c")
            ps6[5] = chain_pool.tile([P, CH], F32, name="psq", tag="c")
            for g in range(HG):
                for m in MORDER:
                    for kj in range(4):
                        nc.tensor.matmul(
                            ps6[m][:],
                            wq_sb[:, g, m, kj, :],
                            hts[g][:, kj, :],
                            start=(g == 0 and kj == 0),
                            stop=(g == HG - 1 and kj == 3),
                        )
            for m in MORDER:
                ps = ps6[m]
                if m == 5:
                    vraw = tmp_pool.tile([P, CH], BF16, name="vraw")
                    nc.scalar.copy(vraw[:], ps[:])
                    psv = pss_pool.tile([P, CH], F32, name="psr", tag="s")
                    psvb = psv.bitcast(BF16)
                    for t in range(4):
                        nc.tensor.transpose(
                            psvb[:, t * P:(t + 1) * P], vraw[:, t * P:(t + 1) * P],
                            id_sb[:],
                        )
                        nc.vector.tensor_copy(
                            vt[:, t, :], psvb[:, t * P:(t + 1) * P]
                        )
                else:
                    raw = tmp_pool.tile([P, CH], BF16, name="raw")
                    nc.scalar.copy(raw[:], ps[:])
                    rope_finish(m, raw)
            qkv_chunk.qp = qp
            return

        pending = None
        for m in MORDER:
            ps = chain_pool.tile([P, CH], F32, name="psq", tag="c")
            for k in range(HT):
                nc.tensor.matmul(
                    ps[:],
                    wq_sb[:, k // 4, m, k % 4, :],
                    hts[k // 4][:, k % 4, :],
                    start=(k == 0),
                    stop=(k == HT - 1),
                )
            if pending is not None:
                rope_finish(*pending)
                pending = None
            if m == 5:
                # V^T chunk -> V tiles [s, d] via PE transpose
                vraw = tmp_pool.tile([P, CH], BF16, name="vraw")
                nc.scalar.copy(vraw[:], ps[:])
                psv = acc_pool.tile([P, CH], F32, name="psv", tag="a")
                psvb = psv.bitcast(BF16)
                for t in range(4):
                    nc.tensor.transpose(
                        psvb[:, t * P:(t + 1) * P], vraw[:, t * P:(t + 1) * P], id_sb[:]
                    )
                    nc.vector.tensor_copy(vt[:, 4 * qc + t, :], psvb[:, t * P:(t + 1) * P])
            else:
                raw = tmp_pool.tile([P, CH], BF16, name="raw")
                nc.scalar.copy(raw[:], ps[:])
                pending = (m, raw)
        qkv_chunk.qp = qp

    ot_cur = [None]

    def attn_chunk(qc, heads=range(QH)):
        nk = 4 * qc + 4
        qp = qkv_chunk.qp
        if heads[0] == 0:
            ot_cur[0] = ot_pool.tile([P, QH, CH], BF16, name="ot")
        ot = ot_cur[0]
        pvs, denbs = [], []
        for h in heads:
            # scores + exp (+ causal mask on diagonal tiles)
            for kj in range(nk):
                v = kj - 4 * qc
                c0 = 128 * v if v >= 0 else 0
                pss = pss_pool.tile([P, CH], F32, name="pss", tag="s")
                nc.tensor.matmul(
                    pss[:, c0:CH],
                    kp[:, kj * P:(kj + 1) * P],
                    qp[:, h, c0:CH],
                    start=True,
                    stop=True,
                )
                nc.scalar.activation(
                    es[:, kj, c0:CH], pss[:, c0:CH],
                    mybir.ActivationFunctionType.Exp,
                    scale=SCALE,
                )
                if v >= 0:
                    nc.vector.tensor_mul(
                        es[:, kj, c0:CH], es[:, kj, c0:CH], tri_sb[:, v, c0:CH]
                    )
            # denominator: pairwise bf16 tree over full es tiles (masked regions
            # are exactly zero, so full-tile sums are correct). bf16 runs 2x the
            # DVE rate of f32; the ~4-level rounding noise (~0.5%) fits budget.
            denb = tmp_pool.tile([P, CH], BF16, name="denb", bufs=QH)

            def tt_add(a, b, final=False):
                if final:
                    nc.vector.tensor_add(denb[:], a, b)
                    return denb[:]
                t = den_pool.tile([P, CH], BF16, name="tt", bufs=6)
                nc.vector.tensor_add(t[:], a, b)
                return t[:]

            # quad-group running ladder: every tt is consumed within <=4 DVE
            # ops of its write (in-order queue safe), ~3 rounding levels
            ngr = nk // 4
            acc = None
            for g in range(ngr):
                l = [es[:, 4 * g + j, :] for j in range(4)]
                p1 = tt_add(l[0], l[1])
                p2 = tt_add(l[2], l[3])
                gsum = tt_add(p1, p2, final=(ngr == 1))
                if g == 0:
                    acc = gsum
                else:
                    acc = tt_add(acc, gsum, final=(g == ngr - 1))
            denbs.append((h, denb))
            # flipped PV: O^T[d, q] accumulated over k tiles (masked es)
            pv = acc_pool.tile([P, CH], F32, name="pv", tag="a")
            for kj in range(nk):
                v = kj - 4 * qc
                c0 = 128 * v if v > 0 else 0
                nc.tensor.matmul(
                    pv[:, c0:CH],
                    vt[:, kj, :],
                    es[:, kj, c0:CH],
                    start=(kj == 0),
                    stop=(kj == nk - 1),
                )
            pvs.append((h, pv))
            if h % 2 == 1:
                # normalize + ship this head pair now (deferred past the pair's
                # PV so the PE never waits the DVE denominator chain), keeping
                # the pair's AllGather as early as possible for o_proj
                pair = h // 2
                dd = dict(denbs)
                pp = dict(pvs)
                for hh in (h - 1, h):
                    dps = chain_pool.tile([P, CH], F32, name="dps", tag="c")
                    nc.tensor.matmul(
                        dps[:], ones_sb[:], dd[hh][:], start=True, stop=True
                    )
                    rec = den_pool.tile([P, CH], F32, name="rec", bufs=1)
                    nc.vector.reciprocal_approx_fast(rec[:], dps[:])
                    nc.vector.tensor_mul(ot[:, hh, :], pp[hh][:], rec[:])
                    nc.gpsimd.dma_start(
                        ag_ins[qc][pair][(hh % 2) * P:(hh % 2 + 1) * P, :],
                        ot[:, hh, :],
                    )
                nc.gpsimd.collective_compute(
                    "AllGather",
                    mybir.AluOpType.bypass,
                    ins=[ag_ins[qc][pair].opt()],
                    outs=[ag_outs[qc][pair].opt()],
                    replica_groups=[list(range(NCORES))],
                )

    ogs_fetched = {}

    def og_fetch(qc):
        # og k-pair stream on the gpsimd(SWDGE) ring. Emitted during the NEXT
        # qkv chunk (ring position after that chunk's AllGather triggers), so
        # all pairs land before oproj(qc) starts and nothing inverts in-order.
        agr_a = ag_outs[qc][0].rearrange("(k p) q -> p k q", p=P)
        agr_b = ag_outs[qc][1].rearrange("(k p) q -> p k q", p=P)
        ogs = []
        for kp2 in range(HT // 2):
            og = og_pool.tile([P, 2, CH], BF16, name="og")
            k2 = 2 * kp2
            src_ap = (agr_a[:, k2:k2 + 2, :] if k2 < KT
                      else agr_b[:, k2 - KT:k2 - KT + 2, :])
            nc.gpsimd.dma_start(og[:], src_ap)
            ogs.append(og[:, 0, :])
            ogs.append(og[:, 1, :])
        ogs_fetched[qc] = ogs

    def oproj_chunk(qc):
        ogs = ogs_fetched[qc]
        psos = [acc_pool.tile([P, WON], F32, name="pso", tag="a") for _ in range(4)]
        for k in range(HT):
            for mi in range(4):
                nc.tensor.matmul(
                    psos[mi][:],
                    ogs[k][:, mi * P:(mi + 1) * P],
                    wo_sb[:, k, :],
                    start=(k == 0),
                    stop=(k == HT - 1),
                )
        for mi in range(4):
            m = qc * 4 + mi
            oev = oev_pool.tile([P, WON], BF16, name="oev")
            nc.vector.tensor_copy(oev[:], psos[mi][:])
            # scalar ring: idle while o_proj runs, so the final chunk's output
            # isn't stuck behind the og trigger burst on the gpsimd ring
            nc.scalar.dma_start(out[m * P:(m + 1) * P, :], oev[:])

    # ---------------- pipelined schedule ----------------
    qkv_chunk(0)
    attn_chunk(0)
    qkv_chunk(1)
    og_fetch(0)
    attn_chunk(1)
    oproj_chunk(0)
    qkv_chunk(2)
    og_fetch(1)
    attn_chunk(2)
    oproj_chunk(1)
    qkv_chunk(3)
    og_fetch(2)
    attn_chunk(3)
    oproj_chunk(2)
    og_fetch(3)
    oproj_chunk(3)

    ctx.close()


def _build():
    if "nc" in _CACHE:
        return _CACHE["nc"]
    nc = bacc.Bacc("TRN2", debug=False, num_devices=NCORES, target_bir_lowering=False)
    aps = {}
    aps["hT"] = nc.dram_tensor("hT", [P, NCH, HT, CH], BF16, kind="ExternalInput").ap()
    aps["wq"] = nc.dram_tensor("wq", [P, HG, 6, 4, P], BF16, kind="ExternalInput").ap()
    aps["wo"] = nc.dram_tensor("wo", [P, HT, WON], BF16, kind="ExternalInput").ap()
    aps["cosT"] = nc.dram_tensor("cosT", [HD, S], BF16, kind="ExternalInput").ap()
    aps["sinT"] = nc.dram_tensor("sinT", [HD, S], BF16, kind="ExternalInput").ap()
    aps["rT"] = nc.dram_tensor("rT", [P, P], BF16, kind="ExternalInput").ap()
    aps["ident"] = nc.dram_tensor("ident", [P, P], BF16, kind="ExternalInput").ap()
    aps["ones"] = nc.dram_tensor("ones", [P, P], BF16, kind="ExternalInput").ap()
    aps["tri"] = nc.dram_tensor("tri", [P, 4, CH], BF16, kind="ExternalInput").ap()
    aps["out"] = nc.dram_tensor("out", [S, WON], BF16, kind="ExternalOutput").ap()
    with tile.TileContext(nc) as tc:
        _emit(nc, tc, aps)
    nc.compile()
    _CACHE["nc"] = nc
    return nc


def _host_tables(positions: np.ndarray):
    pos = np.asarray(positions).reshape(-1).astype(np.float64)
    assert pos.shape[0] == S
    inv = 1.0 / (10000.0 ** (np.arange(0, HD, 2, dtype=np.float64) / HD))  # [64]
    invf = np.concatenate([inv, inv])  # [128], row d uses inv[d % 64]
    th = invf[:, None] * pos[None, :]  # [128, 2048]
    cosT = np.cos(th).astype(NPBF16)
    sinT = np.sin(th).astype(NPBF16)
    R = np.zeros((P, P), np.float32)
    idx = np.arange(64)
    R[idx, idx + 64] = -1.0
    R[idx + 64, idx] = 1.0
    rT = R.T.astype(NPBF16).copy()
    ident = np.eye(P, dtype=NPBF16)
    ones = np.ones((P, P), dtype=NPBF16)
    k_loc = np.arange(P)[:, None]
    q_loc = np.arange(CH)[None, :]
    tri = np.stack(
        [(q_loc >= k_loc + 128 * v) for v in range(4)], axis=1
    ).astype(NPBF16)  # [128, 4, 512]
    return cosT, sinT, rT, ident, ones, tri


def _make_in_maps(inputs: dict):
    hidden = np.asarray(inputs["hidden_states"], np.float32).reshape(S, H)
    positions = np.asarray(inputs["positions"])
    w_qkv = np.asarray(inputs["w_qkv"], np.float32)
    w_o = np.asarray(inputs["w_o"], np.float32)
    cosT, sinT, rT, ident, ones, tri = _host_tables(positions)
    # hT: [4096, 2048] -> [p=128, chunk=4, ktile=32, s=512] (contiguous per partition)
    hT = np.ascontiguousarray(
        hidden.T.astype(NPBF16)
        .reshape(HT, P, NCH, CH)
        .transpose(1, 2, 0, 3)
    )
    in_maps = []
    for c in range(NCORES):
        wqc = np.concatenate([
            w_qkv[:, c * QH * HD:(c + 1) * QH * HD],
            w_qkv[:, NH * HD + c * HD: NH * HD + (c + 1) * HD],
            w_qkv[:, (NH + NKV) * HD + c * HD: (NH + NKV) * HD + (c + 1) * HD],
        ], axis=1)  # [4096, 768] cols: Q0..Q3, K, V
        # -> [p=128, kgroup=8, m=6, kj=4, 128] (k-group-major so chunk-0's
        # group-interleaved chains consume weights in DMA arrival order)
        wq = np.ascontiguousarray(
            wqc.astype(NPBF16).reshape(8, 4, P, 6, P).transpose(2, 0, 3, 1, 4)
        )
        # feature order after the two head-pair AllGathers: ktile k<16 holds
        # (core k//2, head k%2); k>=16 holds (core (k-16)//2, head 2+(k-16)%2)
        heads = [4 * (k // 2) + (k % 2) for k in range(KT)] + \
                [4 * (k // 2) + 2 + (k % 2) for k in range(KT)]
        perm = np.concatenate([h * HD + np.arange(HD) for h in heads])
        wo = np.ascontiguousarray(
            w_o[perm, c * WON:(c + 1) * WON].astype(NPBF16)
            .reshape(HT, P, WON).transpose(1, 0, 2)
        )
        in_maps.append({
            "hT": hT,
            "wq": wq,
            "wo": wo,
            "cosT": cosT,
            "sinT": sinT,
            "rT": rT,
            "ident": ident,
            "ones": ones,
            "tri": tri,
        })
    return in_maps


def _run(inputs: dict, trace: bool = False):
    nc = _build()
    in_maps = _make_in_maps(inputs)
    res = run_bass_kernel_spmd(nc, in_maps, core_ids=list(range(NCORES)), trace=trace)
    full = np.concatenate(
        [np.asarray(res.results[c]["out"]) for c in range(NCORES)], axis=1
    )
    return full.reshape(1, S, H).astype(np.float32), res


def kernel(**inputs) -> np.ndarray:
    out, _ = _run(inputs, trace=False)
    return out


if __name__ == "__main__":
    import sys
    if "--build-only" in sys.argv:
        nc = _build()
        print("build ok; instructions:",
              sum(len(bb.instructions) for bb in nc.main_func.blocks))
